# revision 21
# baseline (speedup 1.0000x reference)
"""Trainium2 Bass kernel for the 3x3 abs-diff stencil module:

    out = x + alpha * sum_{di,dj in 3x3} |x - shift_{di,dj}(zero_pad(x))|

x: (8, 64, 256, 256) f32, alpha: (1, 64, 1, 1) f32.

Strategy (pure data parallel, no collectives; core i <- batch i):

  - Host pre-casts x to bf16 and zero-pads each shard to (64, 258, 260).
  - SBUF layout: partition p = (s, c), s = H-half, c = channel; free dim =
    (rows, cols). All stencil shifts are free-dim AP offsets.
  - DMA: the whole padded input lives in one SBUF tile (67.6 KB/partition)
    filled by 4 big SWDGE loads (~2.3 MB each, 17-18 KB/descriptor) --
    dma cost is ~2us fixed + bytes/436GB/s, so few big transfers beat the
    v1 kernel's 10 small ones by ~3x on queue occupancy.  Stores are
    paired: two 16-row jobs share one [128,32,W] output tile flushed by a
    single ~2.1 MB dma_start.  Everything rides the gpsimd SWDGE queue,
    which dispatches ~3x faster than the HWDGE queues.
  - Per 16-row job, 4 signed diff fields are computed on DVE (bf16 2x):
      dE[t,u]  = xp[t+1,u]   - xp[t+1,u+1]   (horizontal)
      dS[t,w]  = xp[t,w+2]   - xp[t+1,w+2]   (vertical)
      dSE[t,u] = xp[t,u+1]   - xp[t+1,u+2]   (diagonal \\)
      dSW[t,u] = xp[t,u+2]   - xp[t+1,u+1]   (diagonal /)
    abs is split across engines: dE,dS via DVE int32-AND (2 int32/cyc),
    dSE,dSW via ACT Abs.
  - The 8-neighbor sum runs entirely on the PE: for each PSUM bank
    (2 output rows x 256 cols) 8 matmuls accumulate the 8 shifted terms.
    The stationary matrix is diag(alpha) in bf16 for every matmul, so
    PSUM ends up holding alpha * S directly and the drain is a plain ACT
    copy to bf16.
  - DVE then adds x into the drained tile (out = x + alpha*S) and the
    result is stored as bf16 (host casts back to f32).
  - Emission is software-pipelined: ACT abs of job j precede the PSUM
    drains of job j-1; the paired store of jobs (j-3, j-2) is emitted
    inside iteration j so it never head-of-line blocks a load.
"""

import sys

import numpy as np

try:
    import concourse  # noqa: F401
except ImportError:
    sys.path.insert(0, "/opt/trn_rl_repo")

from contextlib import ExitStack

import concourse.bacc as bacc
import concourse.bass as bass
import concourse.mybir as mybir
import concourse.tile as tile
from concourse.bass_utils import run_bass_kernel_spmd

F32 = mybir.dt.float32
BF16 = mybir.dt.bfloat16

C = 64
N_CORES = 8


def build_graph(H=256, W=256):
    """Build the per-core Bass graph (identical on all 8 cores).

    Input DRAM tensor per core: (C, H+2, W+4) bf16 host-padded;
    output (C, H, W) bf16; adiag (128, 128) bf16 = diag(alpha).
    """
    HP, WP = H + 2, W + 4
    HH = H // 2          # rows per half
    assert HH == 128
    # small first jobs prime the pipeline; small last jobs shrink the
    # drain tail.  groups of jobs share one 32-row output tile so stores
    # stay ~1 MB per half.
    jobs = [8, 8, 16, 16, 16, 16, 16, 16, 8, 8]
    groups = [(0, 1, 2), (3, 4), (5, 6), (7, 8), (9,)]
    r0s = [sum(jobs[:i]) for i in range(len(jobs))]

    # load chunks (rows of the padded per-half slab, 130 rows total);
    # per-half 64-partition dma_starts engage all 16 SDMA engines
    # (128-partition [2,C]-outer APs only reach 8 at half rate); the
    # first chunk is small so job 0 can start ~7us in
    load_chunks = [(0, 10), (10, 24), (34, 32), (66, 32), (98, 32)]

    nc = bacc.Bacc("TRN2", target_bir_lowering=False, debug=False,
                   num_devices=N_CORES)
    x_d = nc.dram_tensor("x", [C, HP, WP], BF16, kind="ExternalInput")
    a_d = nc.dram_tensor("adiag", [128, 128], BF16, kind="ExternalInput")
    o_d = nc.dram_tensor("out", [C, H, W], BF16, kind="ExternalOutput")

    sub = mybir.AluOpType.subtract
    Copy = mybir.ActivationFunctionType.Copy
    Abs = mybir.ActivationFunctionType.Abs

    with tile.TileContext(nc) as tc, ExitStack() as ctx:
        const_pool = ctx.enter_context(tc.tile_pool(name="const", bufs=1))
        xp_pool = ctx.enter_context(tc.tile_pool(name="xp", bufs=1))
        d_pool = ctx.enter_context(tc.tile_pool(name="d", bufs=2))
        o_pool = ctx.enter_context(tc.tile_pool(name="o", bufs=2))
        ps_pool = ctx.enter_context(tc.tile_pool(name="ps", bufs=4, space="PSUM"))

        adiag_t = const_pool.tile([128, 128], BF16, name="adiag_t")
        nc.sync.dma_start(out=adiag_t[:], in_=a_d.ap())

        # ---- full-height input tile, 8 per-half loads (~1.1 MB each)
        xp = xp_pool.tile([128, HH + 2, WP], BF16, name="xp", tag="xp")
        pstride = xp.ap[0][0]
        for r0, nr in load_chunks:
            for s in range(2):
                lsrc = bass.AP(x_d, s * HH * WP + r0 * WP,
                               [[HP * WP, C], [1, nr * WP]])
                ldst = bass.AP(xp.tensor,
                               xp.offset + s * C * pstride + r0 * WP,
                               [[pstride, C], [1, nr * WP]])
                nc.gpsimd.dma_start(out=ldst, in_=lsrc)

        def drain_stage(ps_list, o_t, orow, half):
            # ACT: PSUM (= alpha*S, f32) -> bf16 o_t rows; must complete
            # before the next job's matmuls reuse the banks.  Emitted in two
            # halves: the first half goes at the head of the next iteration's
            # ACT stream (its PE deps resolved early in the previous job), so
            # the next job's matmuls aren't gated behind that job's abs ops.
            n = len(ps_list)
            sl = range(0, (n + 1) // 2) if half == 0 else range((n + 1) // 2, n)
            for r in sl:
                ps = ps_list[r]
                nc.scalar.activation(
                    o_t[:, orow + 4 * r:orow + 4 * r + 4, :], ps[:], Copy)

        def final_add(r0, Jj, o_t, orow):
            # DVE: out = alpha*S + x
            nc.vector.tensor_add(o_t[:, orow:orow + Jj, :],
                                 o_t[:, orow:orow + Jj, :],
                                 xp[:, r0 + 1:r0 + Jj + 1, 2:W + 2])

        def store_stage(r0, nrows, o_t, engines=None):
            # per-half SWDGE stores for a group of jobs (~1.05 MB each)
            opstride = o_t.ap[0][0]
            for s in range(2):
                dst = bass.AP(o_d, s * HH * W + r0 * W,
                              [[H * W, C], [1, nrows * W]])
                osrc = bass.AP(o_t.tensor,
                               o_t.offset + s * C * opstride,
                               [[opstride, C], [1, nrows * W]])
                eng = engines[s] if engines else nc.gpsimd
                eng.dma_start(out=dst, in_=osrc)

        def term_matmul(ps, g, d_t, elem_off, row_stride, start, stop):
            # one matmul accumulating one shifted |diff| term (2 rows x 256)
            # into PSUM bank slice g, stationary = diag(alpha)
            mv = bass.AP(d_t.tensor, d_t.offset + elem_off,
                         [[d_t.ap[0][0], 128], [row_stride, 2], [1, W]])
            nc.tensor.matmul(ps[:, 512 * g:512 * g + 512], adiag_t[:], mv,
                             start=start, stop=stop)

        # job -> (group start row, is group head, is group tail)
        jinfo = {}
        for grp in groups:
            for j in grp:
                jinfo[j] = (r0s[grp[0]],
                            j == grp[0], j == grp[-1],
                            r0s[grp[-1]] + jobs[grp[-1]] - r0s[grp[0]])

        pending = None        # (r0, Jj, ps_list, o_t, orow) of job j-1
        fa_pending = None     # (job, r0, Jj, o_t, orow) of job j-2
        store_queue = []      # [(tail_job, r0, nrows, o_t)] awaiting store
        last_fa_job = -1      # highest job whose final_add is emitted
        o_t = None
        for j, (r0, Jj) in enumerate(zip(r0s, jobs)):
            # ---- 4 signed diff fields on DVE (bf16 2x streams); the two
            # ACT-abs fields (dSE,dSW) first so ACT starts earliest
            WD = WP - 2  # 258: diff-tile width
            dSE = d_pool.tile([128, Jj + 1, WD], BF16, name="dSE", tag="dSE")
            dSW = d_pool.tile([128, Jj + 1, WD], BF16, name="dSW", tag="dSW")
            dE = d_pool.tile([128, Jj, WD], BF16, name="dE", tag="dE")
            dS = d_pool.tile([128, Jj + 1, W], BF16, name="dS", tag="dS")

            nc.vector.tensor_tensor(dSE[:], xp[:, r0:r0 + Jj + 1, 1:WD + 1],
                                    xp[:, r0 + 1:r0 + Jj + 2, 2:WD + 2], sub)
            if pending is not None:
                drain_stage(pending[2], pending[3], pending[4], 0)
            if j > 0:
                nc.scalar.activation(dSE[:], dSE[:], Abs)
            nc.vector.tensor_tensor(dSW[:], xp[:, r0:r0 + Jj + 1, 2:WD + 2],
                                    xp[:, r0 + 1:r0 + Jj + 2, 1:WD + 1], sub)
            if j > 0:
                nc.scalar.activation(dSW[:], dSW[:], Abs)
            nc.vector.tensor_tensor(dE[:], xp[:, r0 + 1:r0 + Jj + 1, 0:WD],
                                    xp[:, r0 + 1:r0 + Jj + 1, 1:WD + 1], sub)
            nc.vector.tensor_tensor(dS[:], xp[:, r0:r0 + Jj + 1, 2:W + 2],
                                    xp[:, r0 + 1:r0 + Jj + 2, 2:W + 2], sub)

            # ---- abs: dE,dS in place on DVE (int32 AND clears the packed
            # bf16 sign bits at 2 int32/cycle); for job 0 ALL fields go
            # through the DVE AND so the serial ACT abs chain isn't on the
            # warmup critical path
            and_list = (dE, dS) if j > 0 else (dSE, dSW, dE, dS)
            for dt_ in and_list:
                flat = dt_[:, :, :].rearrange("p r w -> p (r w)")
                flat_i = flat.bitcast(mybir.dt.int32)
                nc.vector.tensor_scalar(flat_i, flat_i, 0x7FFF7FFF, None,
                                        mybir.AluOpType.bitwise_and)

            # ---- pipelined late stages: second drain half of job j-1, then
            # final add of job j-2 (the extra lag keeps DVE's queue head from
            # blocking on j-1's drains ahead of j's subs), then the store of
            # the completed group
            if pending is not None:
                drain_stage(pending[2], pending[3], pending[4], 1)
            if fa_pending is not None:
                final_add(*fa_pending[1:])
                last_fa_job = fa_pending[0]
            fa_pending = (j - 1, pending[0], pending[1], pending[3],
                          pending[4]) if pending is not None else None
            while store_queue and store_queue[0][0] <= last_fa_job:
                store_stage(*store_queue.pop(0)[1:])

            # ---- 8-term accumulate on PE: per bank (2 out rows) 8 matmuls
            # with diag(alpha) stationary
            g0, is_head, is_tail, gro = jinfo[j]
            if is_head:
                o_t = o_pool.tile([128, gro, W], BF16, name="o_t", tag="o")
            orow = r0 - g0
            ps_list = []
            for r in range(Jj // 4):
                ps = ps_pool.tile([128, 4 * W], F32, name="ps", tag="ps")
                ps_list.append(ps)
                for g in range(2):      # one PSUM bank per g (2 rows x 256)
                    rr = 4 * r + 2 * g  # first pixel row (q-coord rr+1)
                    # W@ dE[rr..rr+1, 1:257]   E@ dE[rr..rr+1, 2:258]
                    # N@ dS[rr..rr+1, 0:256]   S@ dS[rr+1..rr+2, 0:256]
                    # NW@dSE[rr..rr+1, 0:256]  SE@dSE[rr+1..rr+2, 1:257]
                    # NE@dSW[rr..rr+1, 1:257]  SW@dSW[rr+1..rr+2, 0:256]
                    terms = (
                        (dE, rr * WD + 1, WD),
                        (dE, rr * WD + 2, WD),
                        (dS, rr * W, W),
                        (dS, (rr + 1) * W, W),
                        (dSE, rr * WD, WD),
                        (dSE, (rr + 1) * WD + 1, WD),
                        (dSW, rr * WD + 1, WD),
                        (dSW, (rr + 1) * WD, WD),
                    )
                    for t, (d_t, off, rstr) in enumerate(terms):
                        term_matmul(ps, g, d_t, off, rstr,
                                    start=(t == 0), stop=(t == len(terms) - 1))
            if is_tail:
                # group complete once j's final_add lands (two iterations on)
                store_queue.append((j, g0, gro, o_t))
            pending = (r0, Jj, ps_list, o_t, orow)

        drain_stage(pending[2], pending[3], pending[4], 0)
        drain_stage(pending[2], pending[3], pending[4], 1)
        if fa_pending is not None:
            final_add(*fa_pending[1:])
        final_add(pending[0], pending[1], pending[3], pending[4])
        # remaining stores; the last tiny one rides the idle HWDGE queues
        for k, (tj, sr0, snr, so_t) in enumerate(store_queue):
            eng = (nc.sync, nc.scalar) if k == len(store_queue) - 1 else None
            store_stage(sr0, snr, so_t, engines=eng)

    nc.compile()
    return nc


def _prep_inputs(x, alpha, H=256, W=256):
    """Shard batch across cores, cast to bf16 and zero-pad on host."""
    import ml_dtypes
    x = np.asarray(x, dtype=np.float32)
    alpha = np.asarray(alpha, dtype=np.float32).reshape(C)
    B = x.shape[0]
    HP, WP = H + 2, W + 4
    adiag = np.zeros((128, 128), dtype=np.float32)
    idx = np.arange(128)
    adiag[idx, idx] = alpha[idx % C]
    adiag = adiag.astype(ml_dtypes.bfloat16)
    in_maps = []
    for i in range(B):
        xs = np.zeros((C, HP, WP), dtype=ml_dtypes.bfloat16)
        xs[:, 1:H + 1, 2:W + 2] = x[i].astype(ml_dtypes.bfloat16)
        in_maps.append({"x": xs, "adiag": adiag})
    return in_maps


_GRAPH_CACHE = {}


def _get_graph(H=256, W=256):
    key = (H, W)
    if key not in _GRAPH_CACHE:
        _GRAPH_CACHE[key] = build_graph(H, W)
    return _GRAPH_CACHE[key]


def kernel(x, alpha, _profile=False):
    x = np.asarray(x, dtype=np.float32)
    alpha = np.asarray(alpha, dtype=np.float32)
    B, c, H, W = x.shape
    assert c == C and B == N_CORES, (B, c, H, W)
    nc = _get_graph(H, W)
    in_maps = _prep_inputs(x, alpha, H, W)
    res = run_bass_kernel_spmd(nc, in_maps, core_ids=list(range(N_CORES)),
                               trace=_profile)
    out = np.stack([res.results[i]["out"].astype(np.float32)
                    for i in range(N_CORES)], axis=0)
    if _profile:
        return out, res
    return out


def kernel_profiled(x, alpha):
    out, res = kernel(x, alpha, _profile=True)
    return out, res.exec_time_ns


# revision 22
# speedup vs baseline: 1.0126x; 1.0126x over previous
"""Trainium2 Bass kernel for the 3x3 abs-diff stencil module:

    out = x + alpha * sum_{di,dj in 3x3} |x - shift_{di,dj}(zero_pad(x))|

x: (8, 64, 256, 256) f32, alpha: (1, 64, 1, 1) f32.

Strategy (pure data parallel, no collectives; core i <- batch i):

  - Host pre-casts x to bf16 and zero-pads each shard to (64, 258, 260).
  - SBUF layout: partition p = (s, c), s = H-half, c = channel; free dim =
    (rows, cols). All stencil shifts are free-dim AP offsets.
  - DMA: the whole padded input lives in one SBUF tile (67.6 KB/partition)
    filled by 4 big SWDGE loads (~2.3 MB each, 17-18 KB/descriptor) --
    dma cost is ~2us fixed + bytes/436GB/s, so few big transfers beat the
    v1 kernel's 10 small ones by ~3x on queue occupancy.  Stores are
    paired: two 16-row jobs share one [128,32,W] output tile flushed by a
    single ~2.1 MB dma_start.  Everything rides the gpsimd SWDGE queue,
    which dispatches ~3x faster than the HWDGE queues.
  - Per 16-row job, 4 signed diff fields are computed on DVE (bf16 2x):
      dE[t,u]  = xp[t+1,u]   - xp[t+1,u+1]   (horizontal)
      dS[t,w]  = xp[t,w+2]   - xp[t+1,w+2]   (vertical)
      dSE[t,u] = xp[t,u+1]   - xp[t+1,u+2]   (diagonal \\)
      dSW[t,u] = xp[t,u+2]   - xp[t+1,u+1]   (diagonal /)
    abs is split across engines: dE,dS via DVE int32-AND (2 int32/cyc),
    dSE,dSW via ACT Abs.
  - The 8-neighbor sum runs entirely on the PE: for each PSUM bank
    (2 output rows x 256 cols) 8 matmuls accumulate the 8 shifted terms.
    The stationary matrix is diag(alpha) in bf16 for every matmul, so
    PSUM ends up holding alpha * S directly and the drain is a plain ACT
    copy to bf16.
  - DVE then adds x into the drained tile (out = x + alpha*S) and the
    result is stored as bf16 (host casts back to f32).
  - Emission is software-pipelined: ACT abs of job j precede the PSUM
    drains of job j-1; the paired store of jobs (j-3, j-2) is emitted
    inside iteration j so it never head-of-line blocks a load.
"""

import sys

import numpy as np

try:
    import concourse  # noqa: F401
except ImportError:
    sys.path.insert(0, "/opt/trn_rl_repo")

from contextlib import ExitStack

import concourse.bacc as bacc
import concourse.bass as bass
import concourse.mybir as mybir
import concourse.tile as tile
from concourse.bass_utils import run_bass_kernel_spmd

F32 = mybir.dt.float32
BF16 = mybir.dt.bfloat16

C = 64
N_CORES = 8


def build_graph(H=256, W=256):
    """Build the per-core Bass graph (identical on all 8 cores).

    Input DRAM tensor per core: (C, H+2, W+4) bf16 host-padded;
    output (C, H, W) bf16; adiag (128, 128) bf16 = diag(alpha).
    """
    HP, WP = H + 2, W + 4
    HH = H // 2          # rows per half
    assert HH == 128
    # small first jobs prime the pipeline; small last jobs shrink the
    # drain tail.  groups of jobs share one 32-row output tile so stores
    # stay ~1 MB per half.
    jobs = [8, 8, 16, 16, 16, 16, 16, 16, 8, 8]
    groups = [(0, 1, 2), (3, 4), (5, 6), (7, 8), (9,)]
    r0s = [sum(jobs[:i]) for i in range(len(jobs))]

    # load chunks (rows of the padded per-half slab, 130 rows total);
    # per-half 64-partition dma_starts engage all 16 SDMA engines
    # (128-partition [2,C]-outer APs only reach 8 at half rate); the
    # first chunk is small so job 0 can start ~7us in
    load_chunks = [(0, 10), (10, 24), (34, 32), (66, 32), (98, 32)]

    nc = bacc.Bacc("TRN2", target_bir_lowering=False, debug=False,
                   num_devices=N_CORES)
    x_d = nc.dram_tensor("x", [C, HP, WP], BF16, kind="ExternalInput")
    a_d = nc.dram_tensor("adiag", [128, 128], BF16, kind="ExternalInput")
    o_d = nc.dram_tensor("out", [C, H, W], BF16, kind="ExternalOutput")

    sub = mybir.AluOpType.subtract
    Copy = mybir.ActivationFunctionType.Copy
    Abs = mybir.ActivationFunctionType.Abs

    with tile.TileContext(nc) as tc, ExitStack() as ctx:
        const_pool = ctx.enter_context(tc.tile_pool(name="const", bufs=1))
        xp_pool = ctx.enter_context(tc.tile_pool(name="xp", bufs=1))
        d_pool = ctx.enter_context(tc.tile_pool(name="d", bufs=2))
        o_pool = ctx.enter_context(tc.tile_pool(name="o", bufs=2))
        ps_pool = ctx.enter_context(tc.tile_pool(name="ps", bufs=4, space="PSUM"))

        adiag_t = const_pool.tile([128, 128], BF16, name="adiag_t")
        nc.sync.dma_start(out=adiag_t[:], in_=a_d.ap())

        # ---- full-height input tile, 8 per-half loads (~1.1 MB each)
        xp = xp_pool.tile([128, HH + 2, WP], BF16, name="xp", tag="xp")
        pstride = xp.ap[0][0]
        for r0, nr in load_chunks:
            for s in range(2):
                lsrc = bass.AP(x_d, s * HH * WP + r0 * WP,
                               [[HP * WP, C], [1, nr * WP]])
                ldst = bass.AP(xp.tensor,
                               xp.offset + s * C * pstride + r0 * WP,
                               [[pstride, C], [1, nr * WP]])
                nc.gpsimd.dma_start(out=ldst, in_=lsrc)

        def drain_stage(ps_list, o_t, orow, half):
            # ACT: PSUM (= alpha*S, f32) -> bf16 o_t rows; must complete
            # before the next job's matmuls reuse the banks.  Emitted in two
            # halves: the first half goes at the head of the next iteration's
            # ACT stream (its PE deps resolved early in the previous job), so
            # the next job's matmuls aren't gated behind that job's abs ops.
            n = len(ps_list)
            sl = range(0, (n + 1) // 2) if half == 0 else range((n + 1) // 2, n)
            for r in sl:
                ps = ps_list[r]
                nc.scalar.activation(
                    o_t[:, orow + 4 * r:orow + 4 * r + 4, :], ps[:], Copy)

        def final_add(r0, Jj, o_t, orow):
            # DVE: out = alpha*S + x
            nc.vector.tensor_add(o_t[:, orow:orow + Jj, :],
                                 o_t[:, orow:orow + Jj, :],
                                 xp[:, r0 + 1:r0 + Jj + 1, 2:W + 2])

        def store_stage(r0, nrows, o_t, engines=None):
            # per-half SWDGE stores for a group of jobs (~1.05 MB each)
            opstride = o_t.ap[0][0]
            for s in range(2):
                dst = bass.AP(o_d, s * HH * W + r0 * W,
                              [[H * W, C], [1, nrows * W]])
                osrc = bass.AP(o_t.tensor,
                               o_t.offset + s * C * opstride,
                               [[opstride, C], [1, nrows * W]])
                eng = engines[s] if engines else nc.gpsimd
                eng.dma_start(out=dst, in_=osrc)

        def term_matmul(ps, g, d_t, elem_off, row_stride, start, stop):
            # one matmul accumulating one shifted |diff| term (2 rows x 256)
            # into PSUM bank slice g, stationary = diag(alpha)
            mv = bass.AP(d_t.tensor, d_t.offset + elem_off,
                         [[d_t.ap[0][0], 128], [row_stride, 2], [1, W]])
            nc.tensor.matmul(ps[:, 512 * g:512 * g + 512], adiag_t[:], mv,
                             start=start, stop=stop)

        # job -> (group start row, is group head, is group tail)
        jinfo = {}
        for grp in groups:
            for j in grp:
                jinfo[j] = (r0s[grp[0]],
                            j == grp[0], j == grp[-1],
                            r0s[grp[-1]] + jobs[grp[-1]] - r0s[grp[0]])

        pending = None        # (r0, Jj, ps_list, o_t, orow) of job j-1
        store_queue = []      # [(tail_job, r0, nrows, o_t)] awaiting store
        last_fa_job = -1      # highest job whose final_add is emitted
        o_t = None
        for j, (r0, Jj) in enumerate(zip(r0s, jobs)):
            # ---- 4 signed diff fields on DVE (bf16 2x streams); the two
            # ACT-abs fields (dSE,dSW) first so ACT starts earliest
            WD = WP - 2  # 258: diff-tile width
            dSE = d_pool.tile([128, Jj + 1, WD], BF16, name="dSE", tag="dSE")
            dSW = d_pool.tile([128, Jj + 1, WD], BF16, name="dSW", tag="dSW")
            dE = d_pool.tile([128, Jj, WD], BF16, name="dE", tag="dE")
            dS = d_pool.tile([128, Jj + 1, W], BF16, name="dS", tag="dS")

            nc.vector.tensor_tensor(dSE[:], xp[:, r0:r0 + Jj + 1, 1:WD + 1],
                                    xp[:, r0 + 1:r0 + Jj + 2, 2:WD + 2], sub)
            if pending is not None:
                drain_stage(pending[2], pending[3], pending[4], 0)
            nc.scalar.activation(dSE[:], dSE[:], Abs)
            nc.vector.tensor_tensor(dSW[:], xp[:, r0:r0 + Jj + 1, 2:WD + 2],
                                    xp[:, r0 + 1:r0 + Jj + 2, 1:WD + 1], sub)
            nc.scalar.activation(dSW[:], dSW[:], Abs)
            nc.vector.tensor_tensor(dE[:], xp[:, r0 + 1:r0 + Jj + 1, 0:WD],
                                    xp[:, r0 + 1:r0 + Jj + 1, 1:WD + 1], sub)
            nc.vector.tensor_tensor(dS[:], xp[:, r0:r0 + Jj + 1, 2:W + 2],
                                    xp[:, r0 + 1:r0 + Jj + 2, 2:W + 2], sub)

            # ---- abs: dE,dS in place on DVE (int32 AND clears the packed
            # bf16 sign bits at 2 int32/cycle)
            for dt_ in (dE, dS):
                flat = dt_[:, :, :].rearrange("p r w -> p (r w)")
                flat_i = flat.bitcast(mybir.dt.int32)
                nc.vector.tensor_scalar(flat_i, flat_i, 0x7FFF7FFF, None,
                                        mybir.AluOpType.bitwise_and)

            # ---- pipelined late stages: second drain half + final add of
            # job j-1, store of the completed group
            if pending is not None:
                drain_stage(pending[2], pending[3], pending[4], 1)
                final_add(pending[0], pending[1], pending[3], pending[4])
                last_fa_job = j - 1
            while store_queue and store_queue[0][0] <= last_fa_job:
                store_stage(*store_queue.pop(0)[1:])

            # ---- 8-term accumulate on PE: per bank (2 out rows) 8 matmuls
            # with diag(alpha) stationary
            g0, is_head, is_tail, gro = jinfo[j]
            if is_head:
                o_t = o_pool.tile([128, gro, W], BF16, name="o_t", tag="o")
            orow = r0 - g0
            ps_list = []
            for r in range(Jj // 4):
                ps = ps_pool.tile([128, 4 * W], F32, name="ps", tag="ps")
                ps_list.append(ps)
                for g in range(2):      # one PSUM bank per g (2 rows x 256)
                    rr = 4 * r + 2 * g  # first pixel row (q-coord rr+1)
                    # W@ dE[rr..rr+1, 1:257]   E@ dE[rr..rr+1, 2:258]
                    # N@ dS[rr..rr+1, 0:256]   S@ dS[rr+1..rr+2, 0:256]
                    # NW@dSE[rr..rr+1, 0:256]  SE@dSE[rr+1..rr+2, 1:257]
                    # NE@dSW[rr..rr+1, 1:257]  SW@dSW[rr+1..rr+2, 0:256]
                    terms = (
                        (dE, rr * WD + 1, WD),
                        (dE, rr * WD + 2, WD),
                        (dS, rr * W, W),
                        (dS, (rr + 1) * W, W),
                        (dSE, rr * WD, WD),
                        (dSE, (rr + 1) * WD + 1, WD),
                        (dSW, rr * WD + 1, WD),
                        (dSW, (rr + 1) * WD, WD),
                    )
                    for t, (d_t, off, rstr) in enumerate(terms):
                        term_matmul(ps, g, d_t, off, rstr,
                                    start=(t == 0), stop=(t == len(terms) - 1))
            if is_tail:
                # group complete once j's final_add lands (two iterations on)
                store_queue.append((j, g0, gro, o_t))
            pending = (r0, Jj, ps_list, o_t, orow)

        drain_stage(pending[2], pending[3], pending[4], 0)
        drain_stage(pending[2], pending[3], pending[4], 1)
        final_add(pending[0], pending[1], pending[3], pending[4])
        # remaining stores; the last tiny one rides the idle HWDGE queues
        for k, (tj, sr0, snr, so_t) in enumerate(store_queue):
            eng = (nc.sync, nc.scalar) if k == len(store_queue) - 1 else None
            store_stage(sr0, snr, so_t, engines=eng)

    nc.compile()
    return nc


def _prep_inputs(x, alpha, H=256, W=256):
    """Shard batch across cores, cast to bf16 and zero-pad on host."""
    import ml_dtypes
    x = np.asarray(x, dtype=np.float32)
    alpha = np.asarray(alpha, dtype=np.float32).reshape(C)
    B = x.shape[0]
    HP, WP = H + 2, W + 4
    adiag = np.zeros((128, 128), dtype=np.float32)
    idx = np.arange(128)
    adiag[idx, idx] = alpha[idx % C]
    adiag = adiag.astype(ml_dtypes.bfloat16)
    in_maps = []
    for i in range(B):
        xs = np.zeros((C, HP, WP), dtype=ml_dtypes.bfloat16)
        xs[:, 1:H + 1, 2:W + 2] = x[i].astype(ml_dtypes.bfloat16)
        in_maps.append({"x": xs, "adiag": adiag})
    return in_maps


_GRAPH_CACHE = {}


def _get_graph(H=256, W=256):
    key = (H, W)
    if key not in _GRAPH_CACHE:
        _GRAPH_CACHE[key] = build_graph(H, W)
    return _GRAPH_CACHE[key]


def kernel(x, alpha, _profile=False):
    x = np.asarray(x, dtype=np.float32)
    alpha = np.asarray(alpha, dtype=np.float32)
    B, c, H, W = x.shape
    assert c == C and B == N_CORES, (B, c, H, W)
    nc = _get_graph(H, W)
    in_maps = _prep_inputs(x, alpha, H, W)
    res = run_bass_kernel_spmd(nc, in_maps, core_ids=list(range(N_CORES)),
                               trace=_profile)
    out = np.stack([res.results[i]["out"].astype(np.float32)
                    for i in range(N_CORES)], axis=0)
    if _profile:
        return out, res
    return out


def kernel_profiled(x, alpha):
    out, res = kernel(x, alpha, _profile=True)
    return out, res.exec_time_ns
